# revision 10
# baseline (speedup 1.0000x reference)
"""DeepAR (2-layer LSTM encoder + LSTM-cell decoder) Trainium2 Bass kernel.

Sharding: pure data parallel, batch 1024 -> 128 per core across 8 cores
(batch 128 == SBUF partition width).

Per-core layout choices:
  - gates in [128 batch, 2048 gate] layout; gate order reordered to
    [i, f, o, g] so one sigmoid covers cols 0:1536 and tanh covers 1536:2048
  - matmuls in bf16 (1 cyc/col on PE), fp32 PSUM accumulation
  - biases injected exactly via K=1 float32r matmuls (ones x bias-row)
  - elementwise (sigmoid/tanh on ACT, mul/add on DVE) in fp32
  - recurrent h produced in bf16, transposed to stationary [K,M] layout via
    DMA-xbar transpose (no PE/PSUM cost)
  - decoder context contribution precomputed once and injected into PSUM via
    identity matmul each step
"""
import numpy as np
import ml_dtypes

import concourse.bass as bass
import concourse.mybir as mybir
import concourse.tile as tile
from concourse import bacc
from concourse.bass_utils import run_bass_kernel_spmd
from concourse.masks import make_identity

F32 = mybir.dt.float32
F32R = mybir.dt.float32r
BF16 = mybir.dt.bfloat16
AF = mybir.ActivationFunctionType

B, T_ENC, H_DEC = 1024, 168, 24
ENC_IN, DEC_IN, HID = 32, 16, 512
G = 4 * HID  # 2048
NCORES = 8
BL = B // NCORES  # 128 batch per core
XCHUNK = 28  # encoder-input steps per DMA chunk

# gate reorder: torch order [i, f, g, o] -> [i, f, o, g]
_PERM = np.concatenate([np.arange(0, 512), np.arange(512, 1024),
                        np.arange(1536, 2048), np.arange(1024, 1536)])


def _bf16(x):
    return np.ascontiguousarray(x.astype(ml_dtypes.bfloat16))


def _f32(x):
    return np.ascontiguousarray(x.astype(np.float32))


def _wT_kxn(W):
    """[4H, D] gate-major weight -> reordered W.T as [128, D//128, 4H] bf16."""
    Wt = W[_PERM].T  # [D, 2048]
    D = Wt.shape[0]
    return _bf16(Wt.reshape(D // 128, 128, G).transpose(1, 0, 2))


def build_kernel(T=T_ENC, HD=H_DEC):
    nc = bacc.Bacc("TRN2", target_bir_lowering=False, debug=False,
                   num_devices=NCORES)

    def din(name, shape, dt):
        return nc.dram_tensor(name, shape, dt, kind="ExternalInput").ap()

    x_d = din("x", [ENC_IN + 1, T, BL], BF16)        # enc features + ones row
    w0_d = din("w0", [ENC_IN + 1, G], BF16)           # W_ih0T + bias row
    wh0_d = din("wh0", [128, 4, G], BF16)
    wi1_d = din("wi1", [128, 4, G], BF16)
    wh1_d = din("wh1", [128, 4, G], BF16)
    wctx_d = din("wctx", [128, 4, G], BF16)
    whd_d = din("whd", [128, 4, G], BF16)
    be_d = din("be", [33, G + 128], BF16)  # row0: b1|ones, row32: bd|ones
    covy_d = din("covy", [DEC_IN + 1, HD, BL], BF16)  # dec covariates + y_prev
    wcy_d = din("wcy", [DEC_IN + 1, G], BF16)
    wms_d = din("wms", [128, 4, 2], BF16)
    bms_d = din("bms", [33, 130], BF16)  # row0: [b_mu,b_sig]+pad, cols128:130; ones in cols 0:128

    mu_d = nc.dram_tensor("mu", [BL, HD], F32, kind="ExternalOutput").ap()
    dbg_d = nc.dram_tensor("dbg", [BL, 4], F32, kind="ExternalOutput").ap()
    sg_d = nc.dram_tensor("sg", [BL, HD], F32, kind="ExternalOutput").ap()

    with tile.TileContext(nc) as tc:
        _emit(tc, T, HD, x_d, w0_d, wh0_d, wi1_d, wh1_d, wctx_d, whd_d,
              be_d, covy_d, wcy_d, wms_d, bms_d, mu_d, sg_d, dbg_d)
    nc.compile()
    return nc


def _emit(tc, T, HD, x_d, w0_d, wh0_d, wi1_d, wh1_d, wctx_d, whd_d,
          be_d, covy_d, wcy_d, wms_d, bms_d, mu_d, sg_d, dbg_d=None):
    nc = tc.nc
    mm = nc.tensor.matmul

    with (
        tc.tile_pool(name="const", bufs=1) as cp,
        tc.tile_pool(name="xp", bufs=2) as xp,
        tc.tile_pool(name="sig", bufs=3) as sigp,
        tc.tile_pool(name="small", bufs=3) as smp,
        tc.tile_pool(name="hp", bufs=2) as hp,
        tc.tile_pool(name="htp", bufs=2) as htp,
        tc.tile_pool(name="psum", bufs=2, space="PSUM") as pp,
    ):
        # ---- persistent tiles / weight loads ----
        def load(name, dram, shape, dt):
            t = cp.tile(shape, dt, tag=name)
            nc.sync.dma_start(t[:], dram[:])
            return t

        w0 = load("w0", w0_d, [ENC_IN + 1, G], BF16)
        wh0 = load("wh0", wh0_d, [128, 4, G], BF16)
        wi1 = load("wi1", wi1_d, [128, 4, G], BF16)
        wh1 = load("wh1", wh1_d, [128, 4, G], BF16)
        wctx = load("wctx", wctx_d, [128, 4, G], BF16)
        whd = load("whd", whd_d, [128, 4, G], BF16)
        be = load("be", be_d, [33, G + 128], BF16)
        covy = load("covy", covy_d, [DEC_IN + 1, HD, BL], BF16)
        wcy = load("wcy", wcy_d, [DEC_IN + 1, G], BF16)
        wms = load("wms", wms_d, [128, 4, 2], BF16)
        bms = load("bms", bms_d, [33, 130], BF16)

        ident = cp.tile([128, 128], BF16, tag="ident")
        make_identity(nc, ident[:])
        ones_r = be[0:1, G:G + 128]
        ones32_r = be[32:33, G:G + 128]
        b1_r = be[0:1, 0:G]
        bd_r = be[32:33, 0:G]
        ones_b = bms[0:1, 0:128]
        bms_r = bms[0:1, 128:130]

        c0 = cp.tile([128, HID], F32, tag="c0")
        c1 = cp.tile([128, HID], F32, tag="c1")
        cd = cp.tile([128, HID], F32, tag="cd")
        mu_b = cp.tile([128, HD], F32, tag="mu_b")
        sp_b = cp.tile([128, HD], F32, tag="sp_b")
        sg_b = cp.tile([128, HD], F32, tag="sg_b")

        NS = G // 512  # 4 n-chunks

        def cell(g, c, first, h_tag):
            """gates psum g -> h (bf16 [128, HID]) via ACT/DVE ops."""
            sig = sigp.tile([128, 3 * HID], F32, tag="sig")
            nc.scalar.activation(sig[:], g[:, 0:3 * HID], AF.Sigmoid)
            tg = smp.tile([128, HID], F32, tag="tg")
            nc.scalar.activation(tg[:], g[:, 3 * HID:G], AF.Tanh)
            if first:
                nc.vector.tensor_mul(c[:], sig[:, 0:HID], tg[:])
            else:
                m1 = smp.tile([128, HID], F32, tag="m1")
                nc.vector.tensor_mul(m1[:], sig[:, 0:HID], tg[:])
                m2 = smp.tile([128, HID], F32, tag="m2")
                nc.vector.tensor_mul(m2[:], sig[:, HID:2 * HID], c[:])
                nc.vector.tensor_add(c[:], m1[:], m2[:])
            tcn = smp.tile([128, HID], F32, tag="tc")
            nc.scalar.activation(tcn[:], c[:], AF.Tanh)
            h = hp.tile([128, HID], BF16, tag=h_tag)
            nc.vector.tensor_mul(h[:], sig[:, 2 * HID:3 * HID], tcn[:])
            return h

        def transp(h, tag):
            ht = htp.tile([128, 4, 128], BF16, tag=tag)
            for k in range(4):
                nc.sync.dma_start(ht[:, k, :], h[:, k * 128:(k + 1) * 128],
                                  transpose=True)
            return ht

        # ================= encoder =================
        h0T = h1T = None
        x_sb = None
        for t in range(T):
            if t % XCHUNK == 0:
                nx = min(XCHUNK, T - t)
                x_sb = xp.tile([ENC_IN + 1, XCHUNK, BL], BF16, tag="x")
                nc.sync.dma_start(x_sb[:, :nx, :], x_d[:, t:t + nx, :])
            ti = t % XCHUNK
            # ---- layer 0 ----
            g0 = pp.tile([128, G], F32, tag="g")
            for n in range(NS):
                mm(g0[:, n * 512:(n + 1) * 512], x_sb[:, ti, :],
                   w0[:, n * 512:(n + 1) * 512], start=True, stop=(t == 0))
            if t > 0:
                for k in range(4):
                    for n in range(NS):
                        mm(g0[:, n * 512:(n + 1) * 512], h0T[:, k, :],
                           wh0[:, k, n * 512:(n + 1) * 512],
                           start=False, stop=(k == 3))
            h0 = cell(g0, c0, t == 0, "h0")
            h0T_new = transp(h0, "h0T")

            # ---- layer 1 ----
            g1 = pp.tile([128, G], F32, tag="g")
            for n in range(NS):
                mm(g1[:, n * 512:(n + 1) * 512], ones_r,
                   b1_r[:, n * 512:(n + 1) * 512], start=True, stop=False)
            for k in range(4):
                for n in range(NS):
                    mm(g1[:, n * 512:(n + 1) * 512], h0T_new[:, k, :],
                       wi1[:, k, n * 512:(n + 1) * 512],
                       start=False, stop=(t == 0 and k == 3))
            if t > 0:
                for k in range(4):
                    for n in range(NS):
                        mm(g1[:, n * 512:(n + 1) * 512], h1T[:, k, :],
                           wh1[:, k, n * 512:(n + 1) * 512],
                           start=False, stop=(k == 3))
            h1 = cell(g1, c1, t == 0, "h1")
            h1T = transp(h1, "h1T")
            h0T = h0T_new

        # ================= decoder =================
        # one-time: ctx_pre = context @ W_ctx.T + (b_ihd + b_hhd)
        cps = pp.tile([128, G], F32, tag="g")
        for n in range(NS):
            mm(cps[:, n * 512:(n + 1) * 512], ones32_r,
               bd_r[:, n * 512:(n + 1) * 512], start=True, stop=False)
        for k in range(4):
            for n in range(NS):
                mm(cps[:, n * 512:(n + 1) * 512], h1T[:, k, :],
                   wctx[:, k, n * 512:(n + 1) * 512],
                   start=False, stop=(k == 3))
        ctxp = cp.tile([128, G], BF16, tag="ctxp")
        nc.scalar.copy(ctxp[:], cps[:])

        hdT = None
        for t in range(HD):
            gd = pp.tile([128, G], F32, tag="g")
            for n in range(NS):
                mm(gd[:, n * 512:(n + 1) * 512], ident[:],
                   ctxp[:, n * 512:(n + 1) * 512], start=True, stop=False)
            for n in range(NS):
                mm(gd[:, n * 512:(n + 1) * 512], covy[:, t, :],
                   wcy[:, n * 512:(n + 1) * 512],
                   start=False, stop=(t == 0))
            if t > 0:
                for k in range(4):
                    for n in range(NS):
                        mm(gd[:, n * 512:(n + 1) * 512], hdT[:, k, :],
                           whd[:, k, n * 512:(n + 1) * 512],
                           start=False, stop=(k == 3))
            hd = cell(gd, cd, t == 0, "hd")
            hdT = transp(hd, "hdT")

            hp_ps = pp.tile([128, 2], F32, tag="g")
            mm(hp_ps[:], ones_b, bms_r, start=True, stop=False)
            for k in range(4):
                mm(hp_ps[:], hdT[:, k, :], wms[:, k, :],
                   start=False, stop=(k == 3))
            nc.scalar.copy(mu_b[:, t:t + 1], hp_ps[:, 0:1])
            nc.scalar.copy(sp_b[:, t:t + 1], hp_ps[:, 1:2])
            if t == 0 and dbg_d is not None:
                dbg = cp.tile([128, 4], F32, tag="dbg")
                nc.vector.tensor_copy(dbg[:, 0:2], hp_ps[:, 0:2])
                nc.vector.tensor_copy(dbg[:, 2:3], hd[:, 0:1])
                nc.vector.tensor_copy(dbg[:, 3:4], hdT[:, 0, 0:1])
                nc.sync.dma_start(dbg_d[:], dbg[:])

        # softplus(x) = ln(exp(x) + 1): Exp and Ln share one ACT table set
        nc.scalar.activation(sp_b[:], sp_b[:], AF.Exp)
        nc.scalar.activation(sg_b[:], sp_b[:], AF.Ln, bias=1.0)
        nc.vector.tensor_scalar_add(sg_b[:], sg_b[:], 1e-6)
        nc.sync.dma_start(mu_d[:], mu_b[:])
        nc.sync.dma_start(sg_d[:], sg_b[:])


def _make_bms(b_mu, b_sig):
    bms = np.zeros((33, 130), np.float32)
    bms[0, 0:128] = 1.0
    bms[0, 128] = b_mu[0]
    bms[0, 129] = b_sig[0]
    return _bf16(bms)


def _make_be(b1, bdv):
    be = np.zeros((33, G + 128), np.float32)
    be[0, :G] = b1
    be[32, :G] = bdv
    be[0, G:] = 1.0
    be[32, G:] = 1.0
    return _bf16(be)


def prep_inputs(inputs, T=T_ENC, HD=H_DEC):
    """Full-batch inputs -> list of per-core input maps (host layout prep)."""
    enc = _f32(np.asarray(inputs["enc_inp"]))[:, :T]
    dec = _f32(np.asarray(inputs["dec_inp"]))[:, :HD]
    tgt = _f32(np.asarray(inputs["tgt"]))[:, :HD]

    W_ih0, W_hh0 = np.asarray(inputs["W_ih0"]), np.asarray(inputs["W_hh0"])
    W_ih1, W_hh1 = np.asarray(inputs["W_ih1"]), np.asarray(inputs["W_hh1"])
    W_ihd, W_hhd = np.asarray(inputs["W_ihd"]), np.asarray(inputs["W_hhd"])
    b0 = _f32(np.asarray(inputs["b_ih0"]) + np.asarray(inputs["b_hh0"]))[_PERM]
    b1 = _f32(np.asarray(inputs["b_ih1"]) + np.asarray(inputs["b_hh1"]))[_PERM]
    bdv = _f32(np.asarray(inputs["b_ihd"]) + np.asarray(inputs["b_hhd"]))[_PERM]
    W_mu, b_mu = np.asarray(inputs["W_mu"]), np.asarray(inputs["b_mu"])
    W_sig, b_sig = np.asarray(inputs["W_sig"]), np.asarray(inputs["b_sig"])

    w0 = np.concatenate([W_ih0[_PERM].T, b0[None, :]], 0)  # [33, 2048]
    shared = {
        "w0": _bf16(w0),
        "wh0": _wT_kxn(W_hh0),
        "wi1": _wT_kxn(W_ih1),
        "wh1": _wT_kxn(W_hh1),
        "wctx": _wT_kxn(W_ihd[:, DEC_IN:DEC_IN + HID]),
        "whd": _wT_kxn(W_hhd),
        "be": _make_be(b1, bdv),
        "wcy": _bf16(np.concatenate(
            [W_ihd[_PERM][:, :DEC_IN].T, W_ihd[_PERM][:, DEC_IN + HID:].T], 0)),
        "wms": _bf16(np.concatenate([W_mu, W_sig], 0).T
                     .reshape(4, 128, 2).transpose(1, 0, 2)),
        "bms": _make_bms(b_mu, b_sig),
    }

    in_maps = []
    for c in range(NCORES):
        sl = slice(c * BL, (c + 1) * BL)
        xe = np.ones((ENC_IN + 1, T, BL), np.float32)
        xe[:ENC_IN] = enc[sl].transpose(2, 1, 0)
        cy = np.zeros((DEC_IN + 1, HD, BL), np.float32)
        cy[:DEC_IN] = dec[sl].transpose(2, 1, 0)
        cy[DEC_IN, 1:] = tgt[sl, :HD - 1].T
        m = dict(shared)
        m["x"] = _bf16(xe)
        m["covy"] = _bf16(cy)
        in_maps.append(m)
    return in_maps


_NC_CACHE = {}


def _get_nc(T=T_ENC, HD=H_DEC):
    key = (T, HD)
    if key not in _NC_CACHE:
        _NC_CACHE[key] = build_kernel(T, HD)
    return _NC_CACHE[key]


def run(inputs, T=T_ENC, HD=H_DEC, **kw):
    nc = _get_nc(T, HD)
    in_maps = prep_inputs(inputs, T, HD)
    res = run_bass_kernel_spmd(nc, in_maps, core_ids=list(range(NCORES)), **kw)
    mu = np.concatenate([res.results[c]["mu"] for c in range(NCORES)], 0)
    run.dbg = [res.results[c].get("dbg") for c in range(NCORES)]
    sg = np.concatenate([res.results[c]["sg"] for c in range(NCORES)], 0)
    return (mu, sg), res


def kernel(**inputs):
    (mu, sg), _ = run(inputs)
    return mu, sg


# revision 12
# speedup vs baseline: 1.6320x; 1.6320x over previous
"""DeepAR (2-layer LSTM encoder + LSTM-cell decoder) Trainium2 Bass kernel.

Sharding: pure data parallel, batch 1024 -> 128 per core across 8 cores
(batch 128 == SBUF partition width).

Per-core layout choices:
  - gates in [128 batch, 2048 gate] layout; gate order reordered to
    [i, f, o, g] so one sigmoid covers cols 0:1536 and tanh covers 1536:2048
  - matmuls in bf16 (1 cyc/col on PE), fp32 PSUM accumulation
  - biases injected exactly via K=1 float32r matmuls (ones x bias-row)
  - elementwise (sigmoid/tanh on ACT, mul/add on DVE) in fp32
  - recurrent h produced in bf16, transposed to stationary [K,M] layout via
    DMA-xbar transpose (no PE/PSUM cost)
  - decoder context contribution precomputed once and injected into PSUM via
    identity matmul each step
"""
import numpy as np
import ml_dtypes

import concourse.bass as bass
import concourse.mybir as mybir
import concourse.tile as tile
from concourse import bacc
from concourse.bass_utils import run_bass_kernel_spmd
from concourse.masks import make_identity

F32 = mybir.dt.float32
F32R = mybir.dt.float32r
BF16 = mybir.dt.bfloat16
AF = mybir.ActivationFunctionType

B, T_ENC, H_DEC = 1024, 168, 24
ENC_IN, DEC_IN, HID = 32, 16, 512
G = 4 * HID  # 2048
NCORES = 8
BL = B // NCORES  # 128 batch per core
XCHUNK = 28  # encoder-input steps per DMA chunk

# gate reorder: torch order [i, f, g, o] -> [i, f, o, g]
_PERM = np.concatenate([np.arange(0, 512), np.arange(512, 1024),
                        np.arange(1536, 2048), np.arange(1024, 1536)])


def _bf16(x):
    return np.ascontiguousarray(x.astype(ml_dtypes.bfloat16))


def _f32(x):
    return np.ascontiguousarray(x.astype(np.float32))


def _wT_kxn(W):
    """[4H, D] gate-major weight -> reordered W.T as [128, D//128, 4H] bf16."""
    Wt = W[_PERM].T  # [D, 2048]
    D = Wt.shape[0]
    return _bf16(Wt.reshape(D // 128, 128, G).transpose(1, 0, 2))


def build_kernel(T=T_ENC, HD=H_DEC):
    nc = bacc.Bacc("TRN2", target_bir_lowering=False, debug=False,
                   num_devices=NCORES)

    def din(name, shape, dt):
        return nc.dram_tensor(name, shape, dt, kind="ExternalInput").ap()

    x_d = din("x", [ENC_IN + 1, T, BL], BF16)        # enc features + ones row
    w0_d = din("w0", [ENC_IN + 1, G], BF16)           # W_ih0T + bias row
    wh0_d = din("wh0", [128, 4, G], BF16)
    wi1_d = din("wi1", [128, 4, G], BF16)
    wh1_d = din("wh1", [128, 4, G], BF16)
    wctx_d = din("wctx", [128, 4, G], BF16)
    whd_d = din("whd", [128, 4, G], BF16)
    be_d = din("be", [33, G + 128], BF16)  # row0: b1|ones, row32: bd|ones
    covy_d = din("covy", [DEC_IN + 1, HD, BL], BF16)  # dec covariates + y_prev
    wcy_d = din("wcy", [DEC_IN + 1, G], BF16)
    wms_d = din("wms", [128, 4, 2], BF16)
    bms_d = din("bms", [33, 130], BF16)  # row0: [b_mu,b_sig]+pad, cols128:130; ones in cols 0:128

    mu_d = nc.dram_tensor("mu", [BL, HD], F32, kind="ExternalOutput").ap()
    dbg_d = nc.dram_tensor("dbg", [BL, 4], F32, kind="ExternalOutput").ap()
    sg_d = nc.dram_tensor("sg", [BL, HD], F32, kind="ExternalOutput").ap()

    with tile.TileContext(nc) as tc:
        _emit(tc, T, HD, x_d, w0_d, wh0_d, wi1_d, wh1_d, wctx_d, whd_d,
              be_d, covy_d, wcy_d, wms_d, bms_d, mu_d, sg_d, dbg_d)
    nc.compile()
    return nc


def _emit(tc, T, HD, x_d, w0_d, wh0_d, wi1_d, wh1_d, wctx_d, whd_d,
          be_d, covy_d, wcy_d, wms_d, bms_d, mu_d, sg_d, dbg_d=None):
    nc = tc.nc
    mm = nc.tensor.matmul

    with (
        tc.tile_pool(name="const", bufs=1) as cp,
        tc.tile_pool(name="xp", bufs=2) as xp,
        tc.tile_pool(name="sig", bufs=3) as sigp,
        tc.tile_pool(name="small", bufs=3) as smp,
        tc.tile_pool(name="hp", bufs=2) as hp,
        tc.tile_pool(name="htp", bufs=3) as htp,
        tc.tile_pool(name="psum", bufs=2, space="PSUM") as pp,
    ):
        # ---- persistent tiles / weight loads ----
        def load(name, dram, shape, dt):
            t = cp.tile(shape, dt, tag=name)
            nc.sync.dma_start(t[:], dram[:])
            return t

        w0 = load("w0", w0_d, [ENC_IN + 1, G], BF16)
        wh0 = load("wh0", wh0_d, [128, 4, G], BF16)
        wi1 = load("wi1", wi1_d, [128, 4, G], BF16)
        wh1 = load("wh1", wh1_d, [128, 4, G], BF16)
        wctx = load("wctx", wctx_d, [128, 4, G], BF16)
        whd = load("whd", whd_d, [128, 4, G], BF16)
        be = load("be", be_d, [33, G + 128], BF16)
        covy = load("covy", covy_d, [DEC_IN + 1, HD, BL], BF16)
        wcy = load("wcy", wcy_d, [DEC_IN + 1, G], BF16)
        wms = load("wms", wms_d, [128, 4, 2], BF16)
        bms = load("bms", bms_d, [33, 130], BF16)

        ident = cp.tile([128, 128], BF16, tag="ident")
        make_identity(nc, ident[:])
        ones_r = be[0:1, G:G + 128]
        ones32_r = be[32:33, G:G + 128]
        b1_r = be[0:1, 0:G]
        bd_r = be[32:33, 0:G]
        ones_b = bms[0:1, 0:128]
        bms_r = bms[0:1, 128:130]

        c0 = cp.tile([128, HID], F32, tag="c0")
        c1 = cp.tile([128, HID], F32, tag="c1")
        cd = cp.tile([128, HID], F32, tag="cd")
        mu_b = cp.tile([128, HD], F32, tag="mu_b")
        sp_b = cp.tile([128, HD], F32, tag="sp_b")
        sg_b = cp.tile([128, HD], F32, tag="sg_b")

        NS = G // 512  # 4 n-chunks

        def cell(g, c, first, h_tag):
            """gates psum g -> h (bf16 [128, HID]) via ACT/DVE ops.

            ACT order chosen so DVE can start as early as possible:
            sigmoid(i), tanh(g) -> m1; sigmoid(f,o) -> m2, c; tanh(c) -> h.
            """
            si = smp.tile([128, HID], F32, tag="si")
            nc.scalar.activation(si[:], g[:, 0:HID], AF.Sigmoid)
            tg = smp.tile([128, HID], F32, tag="tg")
            nc.scalar.activation(tg[:], g[:, 3 * HID:G], AF.Tanh)
            sfo = sigp.tile([128, 2 * HID], F32, tag="sfo")
            nc.scalar.activation(sfo[:], g[:, HID:3 * HID], AF.Sigmoid)
            if first:
                nc.vector.tensor_mul(c[:], si[:], tg[:])
            else:
                m1 = smp.tile([128, HID], F32, tag="m1")
                nc.vector.tensor_mul(m1[:], si[:], tg[:])
                m2 = smp.tile([128, HID], F32, tag="m2")
                nc.vector.tensor_mul(m2[:], sfo[:, 0:HID], c[:])
                nc.vector.tensor_add(c[:], m1[:], m2[:])
            tcn = smp.tile([128, HID], F32, tag="tc")
            nc.scalar.activation(tcn[:], c[:], AF.Tanh)
            h = hp.tile([128, HID], BF16, tag=h_tag)
            nc.vector.tensor_mul(h[:], sfo[:, HID:2 * HID], tcn[:])
            return h

        def transp(h, tag):
            ht = htp.tile([128, 4, 128], BF16, tag=tag)
            for k in range(4):
                nc.sync.dma_start(ht[:, k, :], h[:, k * 128:(k + 1) * 128],
                                  transpose=True)
            return ht

        # ================= encoder =================
        # L1 runs one step behind L0: while L0(t)'s elementwise chain runs
        # on ACT/DVE/DMA, the PE stays busy on L1(t-1)'s matmuls (whose
        # inputs are long ready). This keeps the HAM clock-gate warm.
        h0T_hist = {}
        h1T = None
        x_sb = None

        def layer1(t):
            nonlocal h1T
            g1 = pp.tile([128, G], F32, tag="g")
            for n in range(NS):
                mm(g1[:, n * 512:(n + 1) * 512], ones_r,
                   b1_r[:, n * 512:(n + 1) * 512], start=True, stop=False)
            for k in range(4):
                for n in range(NS):
                    mm(g1[:, n * 512:(n + 1) * 512], h0T_hist[t][:, k, :],
                       wi1[:, k, n * 512:(n + 1) * 512],
                       start=False, stop=(t == 0 and k == 3))
            if t > 0:
                for k in range(4):
                    for n in range(NS):
                        mm(g1[:, n * 512:(n + 1) * 512], h1T[:, k, :],
                           wh1[:, k, n * 512:(n + 1) * 512],
                           start=False, stop=(k == 3))
            h1 = cell(g1, c1, t == 0, "h1")
            h1T = transp(h1, "h1T")

        for t in range(T):
            if t % XCHUNK == 0:
                nx = min(XCHUNK, T - t)
                x_sb = xp.tile([ENC_IN + 1, XCHUNK, BL], BF16, tag="x")
                nc.sync.dma_start(x_sb[:, :nx, :], x_d[:, t:t + nx, :])
            ti = t % XCHUNK
            # ---- layer 0, step t ----
            g0 = pp.tile([128, G], F32, tag="g")
            for n in range(NS):
                mm(g0[:, n * 512:(n + 1) * 512], x_sb[:, ti, :],
                   w0[:, n * 512:(n + 1) * 512], start=True, stop=(t == 0))
            if t > 0:
                for k in range(4):
                    for n in range(NS):
                        mm(g0[:, n * 512:(n + 1) * 512], h0T_hist[t - 1][:, k, :],
                           wh0[:, k, n * 512:(n + 1) * 512],
                           start=False, stop=(k == 3))
            h0 = cell(g0, c0, t == 0, "h0")
            h0T_hist[t] = transp(h0, "h0T")
            # ---- layer 1, step t-1 ----
            if t >= 1:
                layer1(t - 1)
        layer1(T - 1)

        # ================= decoder =================
        # one-time: ctx_pre = context @ W_ctx.T + (b_ihd + b_hhd)
        cps = pp.tile([128, G], F32, tag="g")
        for n in range(NS):
            mm(cps[:, n * 512:(n + 1) * 512], ones32_r,
               bd_r[:, n * 512:(n + 1) * 512], start=True, stop=False)
        for k in range(4):
            for n in range(NS):
                mm(cps[:, n * 512:(n + 1) * 512], h1T[:, k, :],
                   wctx[:, k, n * 512:(n + 1) * 512],
                   start=False, stop=(k == 3))
        ctxp = cp.tile([128, G], BF16, tag="ctxp")
        nc.scalar.copy(ctxp[:], cps[:])

        hdT = None
        for t in range(HD):
            gd = pp.tile([128, G], F32, tag="g")
            for n in range(NS):
                mm(gd[:, n * 512:(n + 1) * 512], ident[:],
                   ctxp[:, n * 512:(n + 1) * 512], start=True, stop=False)
            for n in range(NS):
                mm(gd[:, n * 512:(n + 1) * 512], covy[:, t, :],
                   wcy[:, n * 512:(n + 1) * 512],
                   start=False, stop=(t == 0))
            if t > 0:
                for k in range(4):
                    for n in range(NS):
                        mm(gd[:, n * 512:(n + 1) * 512], hdT[:, k, :],
                           whd[:, k, n * 512:(n + 1) * 512],
                           start=False, stop=(k == 3))
            hd = cell(gd, cd, t == 0, "hd")
            hdT = transp(hd, "hdT")

            hp_ps = pp.tile([128, 2], F32, tag="g")
            mm(hp_ps[:], ones_b, bms_r, start=True, stop=False)
            for k in range(4):
                mm(hp_ps[:], hdT[:, k, :], wms[:, k, :],
                   start=False, stop=(k == 3))
            nc.scalar.copy(mu_b[:, t:t + 1], hp_ps[:, 0:1])
            nc.scalar.copy(sp_b[:, t:t + 1], hp_ps[:, 1:2])
            if t == 0 and dbg_d is not None:
                dbg = cp.tile([128, 4], F32, tag="dbg")
                nc.vector.tensor_copy(dbg[:, 0:2], hp_ps[:, 0:2])
                nc.vector.tensor_copy(dbg[:, 2:3], hd[:, 0:1])
                nc.vector.tensor_copy(dbg[:, 3:4], hdT[:, 0, 0:1])
                nc.sync.dma_start(dbg_d[:], dbg[:])

        # softplus(x) = ln(exp(x) + 1): Exp and Ln share one ACT table set
        nc.scalar.activation(sp_b[:], sp_b[:], AF.Exp)
        nc.scalar.activation(sg_b[:], sp_b[:], AF.Ln, bias=1.0)
        nc.vector.tensor_scalar_add(sg_b[:], sg_b[:], 1e-6)
        nc.sync.dma_start(mu_d[:], mu_b[:])
        nc.sync.dma_start(sg_d[:], sg_b[:])


def _make_bms(b_mu, b_sig):
    bms = np.zeros((33, 130), np.float32)
    bms[0, 0:128] = 1.0
    bms[0, 128] = b_mu[0]
    bms[0, 129] = b_sig[0]
    return _bf16(bms)


def _make_be(b1, bdv):
    be = np.zeros((33, G + 128), np.float32)
    be[0, :G] = b1
    be[32, :G] = bdv
    be[0, G:] = 1.0
    be[32, G:] = 1.0
    return _bf16(be)


def prep_inputs(inputs, T=T_ENC, HD=H_DEC):
    """Full-batch inputs -> list of per-core input maps (host layout prep)."""
    enc = _f32(np.asarray(inputs["enc_inp"]))[:, :T]
    dec = _f32(np.asarray(inputs["dec_inp"]))[:, :HD]
    tgt = _f32(np.asarray(inputs["tgt"]))[:, :HD]

    W_ih0, W_hh0 = np.asarray(inputs["W_ih0"]), np.asarray(inputs["W_hh0"])
    W_ih1, W_hh1 = np.asarray(inputs["W_ih1"]), np.asarray(inputs["W_hh1"])
    W_ihd, W_hhd = np.asarray(inputs["W_ihd"]), np.asarray(inputs["W_hhd"])
    b0 = _f32(np.asarray(inputs["b_ih0"]) + np.asarray(inputs["b_hh0"]))[_PERM]
    b1 = _f32(np.asarray(inputs["b_ih1"]) + np.asarray(inputs["b_hh1"]))[_PERM]
    bdv = _f32(np.asarray(inputs["b_ihd"]) + np.asarray(inputs["b_hhd"]))[_PERM]
    W_mu, b_mu = np.asarray(inputs["W_mu"]), np.asarray(inputs["b_mu"])
    W_sig, b_sig = np.asarray(inputs["W_sig"]), np.asarray(inputs["b_sig"])

    w0 = np.concatenate([W_ih0[_PERM].T, b0[None, :]], 0)  # [33, 2048]
    shared = {
        "w0": _bf16(w0),
        "wh0": _wT_kxn(W_hh0),
        "wi1": _wT_kxn(W_ih1),
        "wh1": _wT_kxn(W_hh1),
        "wctx": _wT_kxn(W_ihd[:, DEC_IN:DEC_IN + HID]),
        "whd": _wT_kxn(W_hhd),
        "be": _make_be(b1, bdv),
        "wcy": _bf16(np.concatenate(
            [W_ihd[_PERM][:, :DEC_IN].T, W_ihd[_PERM][:, DEC_IN + HID:].T], 0)),
        "wms": _bf16(np.concatenate([W_mu, W_sig], 0).T
                     .reshape(4, 128, 2).transpose(1, 0, 2)),
        "bms": _make_bms(b_mu, b_sig),
    }

    in_maps = []
    for c in range(NCORES):
        sl = slice(c * BL, (c + 1) * BL)
        xe = np.ones((ENC_IN + 1, T, BL), np.float32)
        xe[:ENC_IN] = enc[sl].transpose(2, 1, 0)
        cy = np.zeros((DEC_IN + 1, HD, BL), np.float32)
        cy[:DEC_IN] = dec[sl].transpose(2, 1, 0)
        cy[DEC_IN, 1:] = tgt[sl, :HD - 1].T
        m = dict(shared)
        m["x"] = _bf16(xe)
        m["covy"] = _bf16(cy)
        in_maps.append(m)
    return in_maps


_NC_CACHE = {}


def _get_nc(T=T_ENC, HD=H_DEC):
    key = (T, HD)
    if key not in _NC_CACHE:
        _NC_CACHE[key] = build_kernel(T, HD)
    return _NC_CACHE[key]


def run(inputs, T=T_ENC, HD=H_DEC, **kw):
    nc = _get_nc(T, HD)
    in_maps = prep_inputs(inputs, T, HD)
    res = run_bass_kernel_spmd(nc, in_maps, core_ids=list(range(NCORES)), **kw)
    mu = np.concatenate([res.results[c]["mu"] for c in range(NCORES)], 0)
    run.dbg = [res.results[c].get("dbg") for c in range(NCORES)]
    sg = np.concatenate([res.results[c]["sg"] for c in range(NCORES)], 0)
    return (mu, sg), res


def kernel(**inputs):
    (mu, sg), _ = run(inputs)
    return mu, sg
